# revision 5
# baseline (speedup 1.0000x reference)
"""Trainium2 Bass kernel for the DTI predictor (gnn_message_passing).

Math (reference):
  a_mol = mol_feats @ Wmu[:H] + bmu            [N, heads]
  a_pro = fused_feats @ Wmu[H:]                [P, heads]
  y_atom[n,h] = sum_p ( elu(a_mol[n,h] + a_pro[p,h]) + 1 )
  y = segment_sum(y_atom, mol_batch, B) * 1e-3
  out = elu(y @ W1 + b1) @ W2 + b2             [B, 1]

Key identity:   elu(x) + 1 = relu(x) + min(exp(a_mol)*exp(a_pro), 1)
so the inner sum becomes two fused engine passes per [128, P] tile:
  ACT:  relu(bc_a + a_mol)  with accum_out  (sum over P fused)
  DVE:  s = (bc_e * e_mol) min 1  then an add-reduce pass
where bc_a / bc_e are a_pro / exp(a_pro) rows broadcast across the 128
atom partitions (built by a ones-matmul on the TensorEngine + copy).

Sharding: 16 heads split across 8 cores (2 heads each, full N and P).
Each core emits a disjoint [B, 2] slice of the pooled tensor -> host
concat + tiny output MLP. No collectives.
"""

import sys

sys.path.insert(0, "/opt/trn_rl_repo")

import numpy as np
import ml_dtypes

import concourse.bass as bass
import concourse.tile as tile
import concourse.bacc as bacc
from concourse import mybir
from concourse.bass_utils import run_bass_kernel_spmd

N_MOL, P_PRO, HID, HEADS, B = 2048, 2048, 64, 16, 64
N_CORES = 8
HPC = HEADS // N_CORES          # heads per core = 2
NT = N_MOL // 128               # atom partition-tiles = 16
NCH = P_PRO // 512              # 512-col chunks of P = 4
F32 = mybir.dt.float32
BF16 = mybir.dt.bfloat16
ALU = mybir.AluOpType
AF = mybir.ActivationFunctionType

# Tiles whose relu pass runs on ACT (the rest run on DVE). Tunable split
# to balance the two engines; index = t * HPC + h in [0, 32).
ACT_RELU_COUNT = 22


def build():
    nc = bacc.Bacc("TRN2", target_bir_lowering=False, debug=False,
                   num_devices=N_CORES)
    molT_d = nc.dram_tensor("molT", [HID + 1, N_MOL], F32, kind="ExternalInput").ap()
    fusedT_d = nc.dram_tensor("fusedT", [HID, P_PRO], F32, kind="ExternalInput").ap()
    wmol_d = nc.dram_tensor("wmol", [HID + 1, HPC], F32, kind="ExternalInput").ap()
    wpro_d = nc.dram_tensor("wpro", [HID, HPC], F32, kind="ExternalInput").ap()
    masks_d = nc.dram_tensor("masks", [128, NT * B], BF16, kind="ExternalInput").ap()
    out_d = nc.dram_tensor("out", [B, HPC], F32, kind="ExternalOutput").ap()

    with tile.TileContext(nc) as tc:
        with (
            tc.tile_pool(name="const", bufs=1) as cpool,
            tc.tile_pool(name="bc", bufs=1) as bcpool,
            tc.tile_pool(name="work", bufs=3) as wpool,
            tc.tile_pool(name="junk", bufs=2) as jpool,
            tc.tile_pool(name="small", bufs=3) as spool,
            tc.tile_pool(name="ps", bufs=2, space=bass.MemorySpace.PSUM) as pspool,
            tc.tile_pool(name="psacc", bufs=1, space=bass.MemorySpace.PSUM) as accpool,
        ):
            # ---- constants / inputs to SBUF ----
            molT = cpool.tile([HID + 1, N_MOL], F32, tag="molT")
            fusedT = cpool.tile([HID, P_PRO], F32, tag="fusedT")
            wmol = cpool.tile([HID + 1, HPC], F32, tag="wmol")
            wpro = cpool.tile([HID, HPC], F32, tag="wpro")
            masks = cpool.tile([128, NT * B], BF16, tag="masks")
            ones = cpool.tile([1, 128], BF16, tag="ones")
            for j in range(NCH):
                nc.sync.dma_start(fusedT[:, bass.ts(j, 512)], fusedT_d[:, bass.ts(j, 512)])
            for j in range(NCH):
                nc.sync.dma_start(molT[:, bass.ts(j, 512)], molT_d[:, bass.ts(j, 512)])
            nc.sync.dma_start(wmol[:], wmol_d)
            nc.sync.dma_start(wpro[:], wpro_d)
            for j in range(2):
                nc.sync.dma_start(masks[:, bass.ts(j, NT * B // 2)],
                                  masks_d[:, bass.ts(j, NT * B // 2)])
            nc.vector.memset(ones[:], 1.0)

            # ---- a_proT rows, one tile per head (matmul rhs needs base partition 0) ----
            aprow = [cpool.tile([1, P_PRO], BF16, tag=f"aprow{h}", name=f"aprow{h}")
                     for h in range(HPC)]
            for h in range(HPC):
                for j in range(NCH):
                    ap_ps = pspool.tile([1, 512], F32, tag="ap_ps")
                    nc.tensor.matmul(ap_ps[:], wpro[:, h:h + 1],
                                     fusedT[:, bass.ts(j, 512)],
                                     start=True, stop=True)
                    nc.vector.tensor_copy(aprow[h][:, bass.ts(j, 512)], ap_ps[:])

            # ---- broadcast tiles per head: bc_a = a_pro row, bc_e = exp ----
            bc_a = [bcpool.tile([128, P_PRO], BF16, tag=f"bca{h}", name=f"bca{h}") for h in range(HPC)]
            bc_e = [bcpool.tile([128, P_PRO], BF16, tag=f"bce{h}", name=f"bce{h}") for h in range(HPC)]
            for h in range(HPC):
                for j in range(NCH):
                    bc_ps = pspool.tile([128, 512], F32, tag="bc_ps")
                    nc.tensor.matmul(bc_ps[:], ones[:],
                                     aprow[h][:, bass.ts(j, 512)],
                                     start=True, stop=True)
                    nc.vector.tensor_copy(bc_a[h][:, bass.ts(j, 512)], bc_ps[:])
                    nc.scalar.activation(bc_e[h][:, bass.ts(j, 512)], bc_ps[:], AF.Exp)

            # ---- pooled accumulator ----
            pool_ps = accpool.tile([B, HPC], F32, tag="pool_ps")

            # ---- main loop over atom tiles ----
            for t in range(NT):
                am_ps = pspool.tile([128, HPC], F32, tag="am_ps")
                nc.tensor.matmul(am_ps[:], molT[:, bass.ts(t, 128)], wmol[:],
                                 start=True, stop=True)
                am = spool.tile([128, HPC], F32, tag="am")
                nc.vector.tensor_copy(am[:], am_ps[:])
                em = spool.tile([128, HPC], F32, tag="em")
                nc.scalar.activation(em[:], am_ps[:], AF.Exp)

                racc = spool.tile([128, HPC], F32, tag="racc")
                sacc = spool.tile([128, HPC], F32, tag="sacc")
                for h in range(HPC):
                    if t * HPC + h < ACT_RELU_COUNT:
                        rjunk = jpool.tile([128, P_PRO], BF16, tag="rjunk")
                        nc.scalar.activation(rjunk[:], bc_a[h][:], AF.Relu,
                                             bias=am[:, h:h + 1],
                                             accum_out=racc[:, h:h + 1])
                    else:
                        rt = wpool.tile([128, P_PRO], BF16, tag="rt")
                        nc.vector.tensor_scalar(rt[:], bc_a[h][:], am[:, h:h + 1],
                                                0.0, ALU.add, ALU.max)
                        rj = jpool.tile([128, P_PRO], BF16, tag="rj")
                        nc.vector.tensor_scalar(rj[:], rt[:], 0.0, None,
                                                ALU.add, ALU.add,
                                                accum_out=racc[:, h:h + 1])
                    st = wpool.tile([128, P_PRO], BF16, tag="st")
                    nc.vector.tensor_scalar(st[:], bc_e[h][:], em[:, h:h + 1],
                                            1.0, ALU.mult, ALU.min)
                    sj = jpool.tile([128, P_PRO], BF16, tag="sj")
                    nc.vector.tensor_scalar(sj[:], st[:], 0.0, None,
                                            ALU.add, ALU.add,
                                            accum_out=sacc[:, h:h + 1])

                yf = spool.tile([128, HPC], F32, tag="yf")
                nc.vector.tensor_add(yf[:], racc[:], sacc[:])
                yb = spool.tile([128, HPC], BF16, tag="yb")
                nc.vector.tensor_copy(yb[:], yf[:])
                nc.tensor.matmul(pool_ps[:], masks[:, bass.ts(t, B)], yb[:],
                                 start=(t == 0), stop=(t == NT - 1))

            out_sb = spool.tile([B, HPC], F32, tag="out_sb")
            nc.scalar.activation(out_sb[:], pool_ps[:], AF.Copy, scale=0.001)
            nc.sync.dma_start(out_d, out_sb[:])

    nc.compile()
    return nc


_NC = None


def _get_nc():
    global _NC
    if _NC is None:
        _NC = build()
    return _NC


def make_in_maps(mol_feats, fused_feats, Wmu, bmu, mol_batch):
    """Host-side sharding: per-core input dicts."""
    molT = np.concatenate([np.asarray(mol_feats, np.float32).T,
                           np.ones((1, N_MOL), np.float32)], axis=0)
    fusedT = np.ascontiguousarray(np.asarray(fused_feats, np.float32).T)
    molT = np.ascontiguousarray(molT)
    Wmu = np.asarray(Wmu, np.float32)
    bmu = np.asarray(bmu, np.float32)
    mb = np.asarray(mol_batch).astype(np.int64)
    # one-hot mask tiles: masks[i, t*B + b] = (mol_batch[t*128 + i] == b)
    masks = np.zeros((128, NT * B), np.float32)
    for t in range(NT):
        seg = mb[t * 128:(t + 1) * 128]
        masks[np.arange(128), t * B + seg] = 1.0
    masks = masks.astype(ml_dtypes.bfloat16)

    in_maps = []
    for c in range(N_CORES):
        h0 = c * HPC
        wmol = np.ascontiguousarray(
            np.concatenate([Wmu[:HID, h0:h0 + HPC], bmu[None, h0:h0 + HPC]], axis=0))
        wpro = np.ascontiguousarray(Wmu[HID:, h0:h0 + HPC])
        in_maps.append({
            "molT": molT, "fusedT": fusedT,
            "wmol": wmol.astype(np.float32), "wpro": wpro.astype(np.float32),
            "masks": masks,
        })
    return in_maps


def _elu(v):
    return np.where(v > 0, v, np.expm1(v))


def finish(pooled, W1, b1, W2, b2):
    """pooled: [B, HEADS] f32 (already * 1e-3). Tiny output MLP."""
    y = _elu(pooled @ np.asarray(W1, np.float32) + np.asarray(b1, np.float32))
    return (y @ np.asarray(W2, np.float32) + np.asarray(b2, np.float32)).astype(np.float32)


def kernel(mol_feats, fused_feats, Wmu, bmu, W1, b1, W2, b2, mol_batch,
           num_graphs, **_unused):
    nc = _get_nc()
    in_maps = make_in_maps(mol_feats, fused_feats, Wmu, bmu, mol_batch)
    res = run_bass_kernel_spmd(nc, in_maps, core_ids=list(range(N_CORES)))
    pooled = np.concatenate([res.results[c]["out"] for c in range(N_CORES)], axis=1)
    return finish(pooled, W1, b1, W2, b2)


# revision 8
# speedup vs baseline: 1.2209x; 1.2209x over previous
"""Trainium2 Bass kernel for the DTI predictor (gnn_message_passing).

Math (reference):
  a_mol = mol_feats @ Wmu[:H] + bmu            [N, heads]
  a_pro = fused_feats @ Wmu[H:]                [P, heads]
  y_atom[n,h] = sum_p ( elu(a_mol[n,h] + a_pro[p,h]) + 1 )
  y = segment_sum(y_atom, mol_batch, B) * 1e-3
  out = elu(y @ W1 + b1) @ W2 + b2             [B, 1]

Key identity:   elu(x) + 1 = relu(x) + min(exp(a_mol)*exp(a_pro), 1)
so the inner sum becomes two fused engine passes per [128, P] tile:
  ACT:  relu(bc_a + a_mol)  with accum_out  (sum over P fused)
  DVE:  s = (bc_e * e_mol) min 1  then an add-reduce pass
where bc_a / bc_e are a_pro / exp(a_pro) rows broadcast across the 128
atom partitions (built by a ones-matmul on the TensorEngine + copy).

Sharding: 16 heads split across 8 cores (2 heads each, full N and P).
Each core emits a disjoint [B, 2] slice of the pooled tensor -> host
concat + tiny output MLP. No collectives.
"""

import sys

sys.path.insert(0, "/opt/trn_rl_repo")

import numpy as np
import ml_dtypes

import concourse.bass as bass
import concourse.tile as tile
import concourse.bacc as bacc
from concourse import mybir
from concourse.bass_utils import run_bass_kernel_spmd

N_MOL, P_PRO, HID, HEADS, B = 2048, 2048, 64, 16, 64
N_CORES = 8
HPC = HEADS // N_CORES          # heads per core = 2
NT = N_MOL // 128               # atom partition-tiles = 16
NCH = P_PRO // 512              # 512-col chunks of P = 4
F32 = mybir.dt.float32
BF16 = mybir.dt.bfloat16
ALU = mybir.AluOpType
AF = mybir.ActivationFunctionType

# Tiles whose relu pass runs on ACT (the rest run on DVE). Tunable split
# to balance the two engines; index = t * HPC + h in [0, 32).
ACT_RELU_COUNT = 24


def build():
    nc = bacc.Bacc("TRN2", target_bir_lowering=False, debug=False,
                   num_devices=N_CORES)
    molT_d = nc.dram_tensor("molT", [HID + 1, N_MOL], F32, kind="ExternalInput").ap()
    fusedT_d = nc.dram_tensor("fusedT", [HID, P_PRO], F32, kind="ExternalInput").ap()
    wmol_d = nc.dram_tensor("wmol", [HID + 1, HPC], F32, kind="ExternalInput").ap()
    wpro_d = nc.dram_tensor("wpro", [HID, HPC], F32, kind="ExternalInput").ap()
    masks_d = nc.dram_tensor("masks", [128, NT * B], BF16, kind="ExternalInput").ap()
    out_d = nc.dram_tensor("out", [B, HPC], F32, kind="ExternalOutput").ap()

    with tile.TileContext(nc) as tc:
        with (
            tc.tile_pool(name="const", bufs=1) as cpool,
            tc.tile_pool(name="bc", bufs=1) as bcpool,
            tc.tile_pool(name="work", bufs=3) as wpool,
            tc.tile_pool(name="junk", bufs=2) as jpool,
            tc.tile_pool(name="small", bufs=3) as spool,
            tc.tile_pool(name="ps", bufs=2, space=bass.MemorySpace.PSUM) as pspool,
            tc.tile_pool(name="psacc", bufs=1, space=bass.MemorySpace.PSUM) as accpool,
        ):
            # ---- constants / inputs to SBUF ----
            molT = cpool.tile([HID + 1, N_MOL], F32, tag="molT")
            fusedT = cpool.tile([HID, P_PRO], F32, tag="fusedT")
            wmol = cpool.tile([HID + 1, HPC], F32, tag="wmol")
            wpro = cpool.tile([HID, HPC], F32, tag="wpro")
            masks = cpool.tile([128, NT * B], BF16, tag="masks")
            ones = cpool.tile([1, 128], BF16, tag="ones")
            ones_big = cpool.tile([128, P_PRO], BF16, tag="ones_big")
            zeros_big = cpool.tile([128, P_PRO], BF16, tag="zeros_big")
            nc.vector.memset(ones_big[:], 1.0)
            nc.vector.memset(zeros_big[:], 0.0)
            for j in range(NCH):
                nc.sync.dma_start(fusedT[:, bass.ts(j, 512)], fusedT_d[:, bass.ts(j, 512)])
            for j in range(NCH):
                nc.sync.dma_start(molT[:, bass.ts(j, 512)], molT_d[:, bass.ts(j, 512)])
            nc.sync.dma_start(wmol[:], wmol_d)
            nc.sync.dma_start(wpro[:], wpro_d)
            for j in range(2):
                nc.sync.dma_start(masks[:, bass.ts(j, NT * B // 2)],
                                  masks_d[:, bass.ts(j, NT * B // 2)])
            nc.vector.memset(ones[:], 1.0)

            # ---- a_proT rows, one tile per head (matmul rhs needs base partition 0) ----
            aprow = [cpool.tile([1, P_PRO], BF16, tag=f"aprow{h}", name=f"aprow{h}")
                     for h in range(HPC)]
            for h in range(HPC):
                for j in range(NCH):
                    ap_ps = pspool.tile([1, 512], F32, tag="ap_ps")
                    nc.tensor.matmul(ap_ps[:], wpro[:, h:h + 1],
                                     fusedT[:, bass.ts(j, 512)],
                                     start=True, stop=True)
                    nc.vector.tensor_copy(aprow[h][:, bass.ts(j, 512)], ap_ps[:])

            # ---- broadcast tiles per head: bc_a = a_pro row, bc_e = exp ----
            bc_a = [bcpool.tile([128, P_PRO], BF16, tag=f"bca{h}", name=f"bca{h}") for h in range(HPC)]
            bc_e = [bcpool.tile([128, P_PRO], BF16, tag=f"bce{h}", name=f"bce{h}") for h in range(HPC)]
            for h in range(HPC):
                for j in range(NCH):
                    bc_ps = pspool.tile([128, 512], F32, tag="bc_ps")
                    nc.tensor.matmul(bc_ps[:], ones[:],
                                     aprow[h][:, bass.ts(j, 512)],
                                     start=True, stop=True)
                    nc.vector.tensor_copy(bc_a[h][:, bass.ts(j, 512)], bc_ps[:])
                    nc.scalar.activation(bc_e[h][:, bass.ts(j, 512)], bc_ps[:], AF.Exp)

            # ---- pooled accumulator ----
            pool_ps = accpool.tile([B, HPC], F32, tag="pool_ps")

            # ---- main loop over atom tiles ----
            for t in range(NT):
                am_ps = pspool.tile([128, HPC], F32, tag="am_ps")
                nc.tensor.matmul(am_ps[:], molT[:, bass.ts(t, 128)], wmol[:],
                                 start=True, stop=True)
                am = spool.tile([128, HPC], F32, tag="am")
                nc.vector.tensor_copy(am[:], am_ps[:])
                em = spool.tile([128, HPC], F32, tag="em")
                nc.scalar.activation(em[:], am_ps[:], AF.Exp)

                racc = spool.tile([128, HPC], F32, tag="racc")
                sacc = spool.tile([128, HPC], F32, tag="sacc")
                for h in range(HPC):
                    if t * HPC + h < ACT_RELU_COUNT:
                        rjunk = jpool.tile([128, P_PRO], BF16, tag="rjunk")
                        nc.scalar.activation(rjunk[:], bc_a[h][:], AF.Relu,
                                             bias=am[:, h:h + 1],
                                             accum_out=racc[:, h:h + 1])
                    else:
                        rt = wpool.tile([128, P_PRO], BF16, tag="rt")
                        nc.vector.scalar_tensor_tensor(
                            rt[:], bc_a[h][:], am[:, h:h + 1], zeros_big[:],
                            ALU.add, ALU.max, accum_out=racc[:, h:h + 1])
                    st = wpool.tile([128, P_PRO], BF16, tag="st")
                    nc.vector.scalar_tensor_tensor(
                        st[:], bc_e[h][:], em[:, h:h + 1], ones_big[:],
                        ALU.mult, ALU.min, accum_out=sacc[:, h:h + 1])

                yf = spool.tile([128, HPC], F32, tag="yf")
                nc.vector.tensor_add(yf[:], racc[:], sacc[:])
                yb = spool.tile([128, HPC], BF16, tag="yb")
                nc.vector.tensor_copy(yb[:], yf[:])
                nc.tensor.matmul(pool_ps[:], masks[:, bass.ts(t, B)], yb[:],
                                 start=(t == 0), stop=(t == NT - 1))

            out_sb = spool.tile([B, HPC], F32, tag="out_sb")
            nc.scalar.activation(out_sb[:], pool_ps[:], AF.Copy, scale=0.001)
            nc.sync.dma_start(out_d, out_sb[:])

    nc.compile()
    return nc


_NC = None


def _get_nc():
    global _NC
    if _NC is None:
        _NC = build()
    return _NC


def make_in_maps(mol_feats, fused_feats, Wmu, bmu, mol_batch):
    """Host-side sharding: per-core input dicts."""
    molT = np.concatenate([np.asarray(mol_feats, np.float32).T,
                           np.ones((1, N_MOL), np.float32)], axis=0)
    fusedT = np.ascontiguousarray(np.asarray(fused_feats, np.float32).T)
    molT = np.ascontiguousarray(molT)
    Wmu = np.asarray(Wmu, np.float32)
    bmu = np.asarray(bmu, np.float32)
    mb = np.asarray(mol_batch).astype(np.int64)
    # one-hot mask tiles: masks[i, t*B + b] = (mol_batch[t*128 + i] == b)
    masks = np.zeros((128, NT * B), np.float32)
    for t in range(NT):
        seg = mb[t * 128:(t + 1) * 128]
        masks[np.arange(128), t * B + seg] = 1.0
    masks = masks.astype(ml_dtypes.bfloat16)

    in_maps = []
    for c in range(N_CORES):
        h0 = c * HPC
        wmol = np.ascontiguousarray(
            np.concatenate([Wmu[:HID, h0:h0 + HPC], bmu[None, h0:h0 + HPC]], axis=0))
        wpro = np.ascontiguousarray(Wmu[HID:, h0:h0 + HPC])
        in_maps.append({
            "molT": molT, "fusedT": fusedT,
            "wmol": wmol.astype(np.float32), "wpro": wpro.astype(np.float32),
            "masks": masks,
        })
    return in_maps


def _elu(v):
    return np.where(v > 0, v, np.expm1(v))


def finish(pooled, W1, b1, W2, b2):
    """pooled: [B, HEADS] f32 (already * 1e-3). Tiny output MLP."""
    y = _elu(pooled @ np.asarray(W1, np.float32) + np.asarray(b1, np.float32))
    return (y @ np.asarray(W2, np.float32) + np.asarray(b2, np.float32)).astype(np.float32)


def kernel(mol_feats, fused_feats, Wmu, bmu, W1, b1, W2, b2, mol_batch,
           num_graphs, **_unused):
    nc = _get_nc()
    in_maps = make_in_maps(mol_feats, fused_feats, Wmu, bmu, mol_batch)
    res = run_bass_kernel_spmd(nc, in_maps, core_ids=list(range(N_CORES)))
    pooled = np.concatenate([res.results[c]["out"] for c in range(N_CORES)], axis=1)
    return finish(pooled, W1, b1, W2, b2)


# revision 15
# speedup vs baseline: 1.4623x; 1.1977x over previous
"""Trainium2 Bass kernel for the DTI predictor (gnn_message_passing).

Math (reference):
  a_mol = mol_feats @ Wmu[:H] + bmu            [N, heads]
  a_pro = fused_feats @ Wmu[H:]                [P, heads]
  y_atom[n,h] = sum_p ( elu(a_mol[n,h] + a_pro[p,h]) + 1 )
  y = segment_sum(y_atom, mol_batch, B) * 1e-3
  out = elu(y @ W1 + b1) @ W2 + b2             [B, 1]

Key identity:   elu(x) + 1 = relu(x) + min(exp(a_mol)*exp(a_pro), 1)

The exp ("s") part runs in p-on-partition layout: one 4x-mode DVE
dual-op per [128p, N] tile (s = (bcast(exp(a_mol)) * exp(a_pro)[p]) min 1)
and the TensorEngine reduces over p via ones-matmuls accumulating into a
[1, N] PSUM row. The relu ("r") part is split: atom tiles t < A2T run
fused on the Scalar engine (relu(bcast(a_pro) + a_mol[n]) with
accum_out, n-on-partition layout); the remaining atom range runs like
the s part on DVE+PE, with the covered range poisoned to -1e9 in the
broadcast row so relu contributes exactly zero there (no double count).

Sharding: 16 heads across 8 cores (2 each, full N and P). Core output:
"out" [B, 2] (device-pooled ACT-range contributions, already *1e-3) and
"yrow" [2, N] (per-atom row sums from the DVE/PE path); host adds the
segment-sum of yrow, concats head slices, and applies the tiny MLP.
"""

import sys

sys.path.insert(0, "/opt/trn_rl_repo")

import numpy as np
import ml_dtypes

import concourse.bass as bass
import concourse.tile as tile
import concourse.bacc as bacc
from concourse import mybir
from concourse.bass_utils import run_bass_kernel_spmd

N_MOL, P_PRO, HID, HEADS, B = 2048, 2048, 64, 16, 64
N_CORES = 8
HPC = HEADS // N_CORES          # heads per core = 2
NT = N_MOL // 128               # atom partition-tiles = 16
NQ = P_PRO // 128               # protein partition-tiles = 16
NCH = P_PRO // 512              # 512-col chunks = 4
F32 = mybir.dt.float32
BF16 = mybir.dt.bfloat16
I32 = mybir.dt.int32
ALU = mybir.AluOpType
AF = mybir.ActivationFunctionType

A2T = 10                        # atom-tiles per head with relu on ACT
RSTART = 1024                   # p-layout r covers n in [RSTART, N); must be
                                # 512-aligned and <= A2T*128 (poison covers rest)
RW = N_MOL - RSTART             # r dual width
RCH0 = RSTART // 512


def build():
    assert RSTART <= A2T * 128 and RSTART % 512 == 0
    nc = bacc.Bacc("TRN2", target_bir_lowering=False, debug=False,
                   num_devices=N_CORES)
    molT_d = nc.dram_tensor("molT", [HID + 1, N_MOL], BF16, kind="ExternalInput").ap()
    fusedT_d = nc.dram_tensor("fusedT", [HID, P_PRO], BF16, kind="ExternalInput").ap()
    wmol_d = nc.dram_tensor("wmol", [HID + 1, HPC], BF16, kind="ExternalInput").ap()
    wpro_d = nc.dram_tensor("wpro", [HID, HPC], BF16, kind="ExternalInput").ap()
    masks_d = nc.dram_tensor("masks", [128, A2T * B], BF16, kind="ExternalInput").ap()
    out_d = nc.dram_tensor("out", [B, HPC], F32, kind="ExternalOutput").ap()
    yrow_d = nc.dram_tensor("yrow", [HPC, N_MOL], F32, kind="ExternalOutput").ap()

    with tile.TileContext(nc) as tc:
        with (
            tc.tile_pool(name="const", bufs=1) as cpool,
            tc.tile_pool(name="bc", bufs=2) as bcpool,
            tc.tile_pool(name="cols", bufs=NT) as colpool,
            tc.tile_pool(name="rows", bufs=1) as rowpool,
            tc.tile_pool(name="work", bufs=4) as wpool,
            tc.tile_pool(name="junk", bufs=2) as jpool,
            tc.tile_pool(name="small", bufs=4) as spool,
            tc.tile_pool(name="ps", bufs=1, space=bass.MemorySpace.PSUM) as pspool,
            tc.tile_pool(name="pssm", bufs=1, space=bass.MemorySpace.PSUM) as smpool,
            tc.tile_pool(name="psrow", bufs=1, space=bass.MemorySpace.PSUM) as rwpool,
            tc.tile_pool(name="psacc", bufs=1, space=bass.MemorySpace.PSUM) as accpool,
        ):
            # ---- inputs ----
            molT = cpool.tile([HID + 1, N_MOL], BF16, tag="molT")
            fusedT = cpool.tile([HID, P_PRO], BF16, tag="fusedT")
            wmol = cpool.tile([HID + 1, HPC], BF16, tag="wmol")
            wpro = cpool.tile([HID, HPC], BF16, tag="wpro")
            masks = cpool.tile([128, A2T * B], BF16, tag="masks")
            for j in range(NCH):
                nc.sync.dma_start(fusedT[:, bass.ts(j, 512)], fusedT_d[:, bass.ts(j, 512)])
            for j in range(NCH):
                nc.sync.dma_start(molT[:, bass.ts(j, 512)], molT_d[:, bass.ts(j, 512)])
            nc.sync.dma_start(wmol[:], wmol_d)
            nc.sync.dma_start(wpro[:], wpro_d)
            nc.sync.dma_start(masks[:], masks_d)

            # ---- constants: ones column, head-select tiles, f32 identity ----
            ones = cpool.tile([128, 1], BF16, tag="ones")
            nc.vector.memset(ones[:], 1.0)
            # sel[h]: [HPC, 128] with row h all-ones -> ones-matmul broadcasts
            # row h of a [HPC, N] row-pair without slicing its partition base.
            iota_p2 = cpool.tile([HPC, 128], F32, tag="iota_p2")
            nc.gpsimd.iota(iota_p2[:], pattern=[[0, 128]], base=0,
                           channel_multiplier=1,
                           allow_small_or_imprecise_dtypes=True)
            sel = []
            for h in range(HPC):
                s = cpool.tile([HPC, 128], BF16, tag=f"sel{h}", name=f"sel{h}")
                nc.vector.tensor_scalar(s[:], iota_p2[:], float(h), None,
                                        ALU.is_equal, ALU.bypass)
                sel.append(s)
            iota_f = cpool.tile([128, 128], F32, tag="iota_f")
            nc.gpsimd.iota(iota_f[:], pattern=[[1, 128]], base=0, channel_multiplier=0,
                           allow_small_or_imprecise_dtypes=True)
            pidx = cpool.tile([128, 1], F32, tag="pidx")
            nc.gpsimd.iota(pidx[:], pattern=[[1, 1]], base=0, channel_multiplier=1,
                           allow_small_or_imprecise_dtypes=True)
            ident = cpool.tile([128, 128], F32, tag="ident")
            nc.vector.tensor_scalar(ident[:], iota_f[:], pidx[:], None,
                                    ALU.is_equal, ALU.bypass)

            # ---- per-tile columns: a_mol ("am"), a_pro ("apc"), exp(a_pro) ("epc") ----
            am = []
            for t in range(NT):
                am_ps = smpool.tile([128, HPC], F32, tag="am_ps")
                nc.tensor.matmul(am_ps[:], molT[:, bass.ts(t, 128)], wmol[:],
                                 start=True, stop=True)
                amt = colpool.tile([128, HPC], F32, tag="am", name=f"am{t}")
                nc.vector.tensor_copy(amt[:], am_ps[:])
                am.append(amt)
            apc, epc = [], []
            for q in range(NQ):
                ap_ps = smpool.tile([128, HPC], F32, tag="ap_ps")
                nc.tensor.matmul(ap_ps[:], fusedT[:, bass.ts(q, 128)], wpro[:],
                                 start=True, stop=True)
                a = colpool.tile([128, HPC], F32, tag="apc", name=f"apc{q}")
                nc.vector.tensor_copy(a[:], ap_ps[:])
                e = colpool.tile([128, HPC], F32, tag="epc", name=f"epc{q}")
                nc.scalar.activation(e[:], ap_ps[:], AF.Exp)
                apc.append(a)
                epc.append(e)

            # ---- rows via PE transposes: aprow/amrow/emrow [HPC, N] ----
            aprow = rowpool.tile([HPC, P_PRO], BF16, tag="aprow")
            row_ps = rwpool.tile([HPC, N_MOL], F32, tag="bigrow", name="row_ps_ap")
            for q in range(NQ):
                nc.tensor.transpose(row_ps[:, bass.ts(q, 128)], apc[q][:], ident[:])
            nc.vector.tensor_copy(aprow[:], row_ps[:])

            amrow = rowpool.tile([HPC, N_MOL], BF16, tag="amrow")
            emrow = rowpool.tile([HPC, N_MOL], BF16, tag="emrow")
            row_ps2 = rwpool.tile([HPC, N_MOL], F32, tag="bigrow", name="row_ps_am")
            for t in range(NT):
                nc.tensor.transpose(row_ps2[:, bass.ts(t, 128)], am[t][:], ident[:])
            nc.scalar.activation(emrow[:], row_ps2[:], AF.Exp)
            nc.vector.tensor_copy(amrow[:], row_ps2[:])
            # poison the ACT-covered atom range so p-layout relu contributes 0
            nc.vector.memset(amrow[:, 0:A2T * 128], -1e9)

            # ---- broadcast tiles per head ----
            bc_a, bc_m, bc_e = [], [], []
            for h in range(HPC):
                bc_a.append(bcpool.tile([128, P_PRO], BF16, tag="bca", name=f"bca{h}"))
                bc_m.append(bcpool.tile([128, RW], BF16, tag="bcm", name=f"bcm{h}"))
                bc_e.append(bcpool.tile([128, N_MOL], BF16, tag="bce", name=f"bce{h}"))

            def build_bc(dst, src_rows, h, col0, ncols):
                """dst[:, :] = broadcast of src_rows[h, col0:col0+ncols]."""
                for j in range(ncols // 512):
                    bc_ps = pspool.tile([128, 512], F32, tag="bc_ps")
                    nc.tensor.matmul(bc_ps[:], sel[h][:],
                                     src_rows[:, col0 + j * 512:col0 + (j + 1) * 512],
                                     start=True, stop=True)
                    nc.vector.tensor_copy(dst[:, bass.ts(j, 512)], bc_ps[:])

            for h in range(HPC):
                build_bc(bc_a[h], aprow, h, 0, P_PRO)
                build_bc(bc_e[h], emrow, h, 0, N_MOL)
                build_bc(bc_m[h], amrow, h, RSTART, RW)

            # ---- pooled accumulator for ACT-range columns ----
            pool_ps = accpool.tile([B, HPC], F32, tag="pool_ps")
            n_pool = A2T * HPC
            pool_i = 0

            # ---- main loops: heads sequential (one PSUM row accumulator) ----
            for h in range(HPC):
                yrow_ps = rwpool.tile([1, N_MOL], F32, tag="bigrow", name=f"yrow{h}")
                for q in range(NQ):
                    st = wpool.tile([128, N_MOL], BF16, tag="st")
                    nc.vector.tensor_scalar(st[:], bc_e[h][:], epc[q][:, h:h + 1],
                                            1.0, ALU.mult, ALU.min)
                    for c in range(NCH):
                        nc.tensor.matmul(yrow_ps[:, bass.ts(c, 512)], ones[:],
                                         st[:, bass.ts(c, 512)],
                                         start=(q == 0),
                                         stop=(q == NQ - 1 and c < RCH0))
                    rt = wpool.tile([128, RW], BF16, tag="rt")
                    nc.vector.tensor_scalar(rt[:], bc_m[h][:], apc[q][:, h:h + 1],
                                            0.0, ALU.add, ALU.max)
                    for c in range(RCH0, NCH):
                        nc.tensor.matmul(yrow_ps[:, bass.ts(c, 512)], ones[:],
                                         rt[:, bass.ts(c - RCH0, 512)],
                                         start=False, stop=(q == NQ - 1))
                # ACT relu for the covered atom range (n-layout, fused accum)
                for t in range(A2T):
                    rjunk = jpool.tile([128, P_PRO], BF16, tag="rjunk")
                    racc = spool.tile([128, 1], F32, tag="racc")
                    nc.scalar.activation(rjunk[:], bc_a[h][:], AF.Relu,
                                         bias=am[t][:, h:h + 1], accum_out=racc[:])
                    rb = spool.tile([128, 1], BF16, tag="rb")
                    nc.vector.tensor_copy(rb[:], racc[:])
                    nc.tensor.matmul(pool_ps[:, h:h + 1], masks[:, bass.ts(t, B)],
                                     rb[:], start=(pool_i == 0),
                                     stop=(pool_i == n_pool - 1))
                    pool_i += 1
                # drain the row accumulator to SBUF then DRAM
                yrow_sb = spool.tile([1, N_MOL], F32, tag="yrow_sb")
                nc.vector.tensor_copy(yrow_sb[:], yrow_ps[:])
                nc.sync.dma_start(yrow_d[h:h + 1, :], yrow_sb[:])

            out_sb = spool.tile([B, HPC], F32, tag="out_sb")
            nc.scalar.activation(out_sb[:], pool_ps[:], AF.Copy, scale=0.001)
            nc.sync.dma_start(out_d, out_sb[:])

    nc.compile()
    return nc


_NC = None


def _get_nc():
    global _NC
    if _NC is None:
        _NC = build()
    return _NC


def make_in_maps(mol_feats, fused_feats, Wmu, bmu, mol_batch):
    """Host-side sharding: per-core input dicts."""
    bf = ml_dtypes.bfloat16
    molT = np.concatenate([np.asarray(mol_feats, np.float32).T,
                           np.ones((1, N_MOL), np.float32)], axis=0)
    molT = np.ascontiguousarray(molT).astype(bf)
    fusedT = np.ascontiguousarray(np.asarray(fused_feats, np.float32).T).astype(bf)
    Wmu = np.asarray(Wmu, np.float32)
    bmu = np.asarray(bmu, np.float32)
    mb = np.asarray(mol_batch).astype(np.int64)
    masks = np.zeros((128, A2T * B), np.float32)
    for t in range(A2T):
        seg = mb[t * 128:(t + 1) * 128]
        masks[np.arange(128), t * B + seg] = 1.0
    masks = masks.astype(bf)

    in_maps = []
    for c in range(N_CORES):
        h0 = c * HPC
        wmol = np.ascontiguousarray(
            np.concatenate([Wmu[:HID, h0:h0 + HPC], bmu[None, h0:h0 + HPC]],
                           axis=0)).astype(bf)
        wpro = np.ascontiguousarray(Wmu[HID:, h0:h0 + HPC]).astype(bf)
        in_maps.append({
            "molT": molT, "fusedT": fusedT,
            "wmol": wmol, "wpro": wpro, "masks": masks,
        })
    return in_maps


def _elu(v):
    return np.where(v > 0, v, np.expm1(v))


def combine(results, mol_batch):
    """Per-core outputs -> pooled [B, HEADS] f32 (already * 1e-3)."""
    mb = np.asarray(mol_batch).astype(np.int64)
    pooled = np.zeros((B, HEADS), np.float32)
    for c in range(N_CORES):
        h0 = c * HPC
        pooled[:, h0:h0 + HPC] += results[c]["out"]
        yrow = results[c]["yrow"]          # [HPC, N] f32
        for h in range(HPC):
            pooled[:, h0 + h] += 1e-3 * np.bincount(
                mb, weights=yrow[h].astype(np.float64), minlength=B
            ).astype(np.float32)
    return pooled


def finish(pooled, W1, b1, W2, b2):
    y = _elu(pooled @ np.asarray(W1, np.float32) + np.asarray(b1, np.float32))
    return (y @ np.asarray(W2, np.float32) + np.asarray(b2, np.float32)).astype(np.float32)


def kernel(mol_feats, fused_feats, Wmu, bmu, W1, b1, W2, b2, mol_batch,
           num_graphs, **_unused):
    nc = _get_nc()
    in_maps = make_in_maps(mol_feats, fused_feats, Wmu, bmu, mol_batch)
    res = run_bass_kernel_spmd(nc, in_maps, core_ids=list(range(N_CORES)))
    pooled = combine(res.results, mol_batch)
    return finish(pooled, W1, b1, W2, b2)
